# revision 9
# baseline (speedup 1.0000x reference)
"""BotRGCN Trainium2 kernel: 8-core SPMD bass kernel.

Strategy (graph/data parallel, per sharding hint):
- Nodes sharded: core c owns dst nodes [c*6250, (c+1)*6250).
- Multi-modal projection: each core computes x0 for its own nodes
  (des/tweets in bf16 via DMA-transpose + matmul, num/cat fp32).
- AllGather x across the 8 cores -> full node-feature table in DRAM.
- RGCN layer: edges (partitioned by dst) sorted by 32-dst windows;
  per 128-edge chunk: indirect-DMA gather of source rows, one-hot
  (iota==code) B matrix on DVE, TensorE matmul G^T.B accumulating
  aggT[fi, 64] per window (both relations side by side), scaled by
  host-precomputed 1/deg, then per-window epilogue matmuls
  (W_r^T @ aggT_r + root^T @ x_ownT) + leaky-relu -> x_{l+1}^T.
- MLP head on each core's own slice; host concatenates [N,2] output.
"""

import os
import numpy as np

NC = 8
N = 50000
S = 6250          # dst nodes per core
S_PAD = 6272      # = 49*128
NTBL = NC * S_PAD  # 50176
D = 128
W = 32            # dsts per aggregation window
NWIN = S_PAD // W  # 196
E = 1600000
DES = 768
TWE = 768
NUMF = 5
CATF = 6
P = 128
ZERO_ROW = S      # row 6250 of slice 0 is a zero pad row (global row index S)

LAST_EXEC_NS = None

_CACHE = {}


def _preprocess(edge_index, edge_type):
    """Sort/pad edges per core into fixed chunk structure shared by all cores.

    Returns (chunks, per_core) where chunks[w] = number of 128-edge chunks for
    window w (same for every core = max over cores), and per_core is a list of
    dicts with offs [128, ncalls] i32, codes [128, ncalls] f32,
    invc [1, NWIN*2*W] f32.
    """
    src = np.asarray(edge_index[0], dtype=np.int64)
    dst = np.asarray(edge_index[1], dtype=np.int64)
    et = np.asarray(edge_type, dtype=np.int64)

    core = dst // S
    dloc = dst - core * S
    win = dloc // W
    code = (dloc - win * W) + W * et          # 0..63
    gsrc = (src // S) * S_PAD + (src % S)     # global table row of src

    per_core_data = []
    cnts = np.zeros((NC, NWIN), dtype=np.int64)
    for c in range(NC):
        m = core == c
        w_c = win[m]
        g_c = gsrc[m]
        cd_c = code[m]
        d_c = dloc[m]
        r_c = et[m]
        order = np.lexsort((g_c, w_c))
        w_c, g_c, cd_c = w_c[order], g_c[order], cd_c[order]
        d_c, r_c = d_c[order], r_c[order]
        cnt = np.bincount(w_c, minlength=NWIN)
        cnts[c] = cnt
        # per (dloc, rel) in-degree for mean
        deg = np.bincount(d_c * 2 + r_c, minlength=S * 2).reshape(S, 2)
        per_core_data.append((g_c, cd_c, d_c, r_c, cnt, deg))

    chunks = ((cnts.max(axis=0) + 127) // 128).astype(np.int64)
    ncalls = int(chunks.sum())
    starts = np.concatenate([[0], np.cumsum(chunks)])

    per_core = []
    for c in range(NC):
        g_c, cd_c, d_c, r_c, cnt, deg = per_core_data[c]
        inv_e = (1.0 / np.maximum(deg[d_c, r_c], 1.0)).astype(np.float32)
        offs = np.full(ncalls * P, ZERO_ROW, dtype=np.int32)
        cods = np.zeros(ncalls * P, dtype=np.float32)
        invs = np.zeros(ncalls * P, dtype=np.float32)
        epos = np.concatenate([[0], np.cumsum(cnt)])
        for w in range(NWIN):
            n = int(cnt[w])
            if n == 0:
                continue
            base = int(starts[w]) * P
            offs[base : base + n] = g_c[epos[w] : epos[w + 1]]
            cods[base : base + n] = cd_c[epos[w] : epos[w + 1]]
            invs[base : base + n] = inv_e[epos[w] : epos[w + 1]]
        # [ncalls*P] -> [P, ncalls]: call k partition p = element k*P+p
        offs = offs.reshape(ncalls, P).T.copy()
        cods = cods.reshape(ncalls, P).T.copy()
        invs = invs.reshape(ncalls, P).T.copy()
        per_core.append(dict(offs=offs, cods=cods, invs=invs))
    return tuple(int(x) for x in chunks), per_core


def _build_program(chunks):
    import concourse.bass as bass
    import concourse.bacc as bacc
    import concourse.mybir as mybir
    import concourse.tile as tile

    f32 = mybir.dt.float32
    bf16 = mybir.dt.bfloat16
    i32 = mybir.dt.int32
    AF = mybir.ActivationFunctionType
    OP = mybir.AluOpType

    ncalls = int(sum(chunks))

    nc = bacc.Bacc("TRN2", target_bir_lowering=False, debug=False, num_devices=NC)

    def din(name, shape, dt):
        return nc.dram_tensor(name, shape, dt, kind="ExternalInput").ap()

    des_i = din("des_i", [S_PAD, DES], bf16)
    tw_i = din("tw_i", [S_PAD, TWE], bf16)
    numT_i = din("numT_i", [NUMF, S_PAD], f32)
    catT_i = din("catT_i", [CATF, S_PAD], f32)
    desw_i = din("desw_i", [DES, D], bf16)
    tww_i = din("tww_i", [TWE, D], bf16)
    numw_i = din("numw_i", [NUMF, D], f32)
    catw_i = din("catw_i", [CATF, D], f32)
    pbT_i = din("pbT_i", [D, 4], f32)          # proj biases per modality
    relw_i = din("relw_i", [2 * 2 * D, D], f32)  # (l,r) blocks of [fi,fo]
    rootw_i = din("rootw_i", [2 * D, D], f32)
    rgbT_i = din("rgbT_i", [D, 2], f32)
    m1w_i = din("m1w_i", [D, D], f32)
    m1bT_i = din("m1bT_i", [D, 1], f32)
    m2w_i = din("m2w_i", [D, 2], f32)
    m2bT_i = din("m2bT_i", [2, 1], f32)
    offs_i = din("offs_i", [P, ncalls], i32)
    cods_i = din("cods_i", [P, ncalls], f32)
    invs_i = din("invs_i", [P, ncalls], f32)
    iota_i = din("iota_i", [P, 2 * W], f32)
    ident_i = din("ident_i", [P, P], f32)

    logitsT_o = nc.dram_tensor("logitsT_o", [2, S_PAD], f32, kind="ExternalOutput").ap()

    # node tile widths for projection / MLP
    widths = [(i * 512, 512) for i in range(12)] + [(12 * 512, 128)]

    with tile.TileContext(nc) as tc:
        with (
            tc.tile_pool(name="const", bufs=1) as cp,
            tc.tile_pool(name="work", bufs=2) as wp,
            tc.tile_pool(name="psum", bufs=1, space="PSUM") as pp,
            tc.tile_pool(name="dram", bufs=1, space="DRAM") as dp,
        ):
            # ---- constants to SBUF ----
            def cload(ap_in, shape, dt, name):
                t = cp.tile(shape, dt, name=name)
                nc.sync.dma_start(out=t[:], in_=ap_in)
                return t

            desw_c = [cload(desw_i[k * P : (k + 1) * P, :], [P, D], bf16, f"desw{k}") for k in range(6)]
            tww_c = [cload(tww_i[k * P : (k + 1) * P, :], [P, D], bf16, f"tww{k}") for k in range(6)]
            numw_c = cload(numw_i[:], [NUMF, D], f32, "numw")
            catw_c = cload(catw_i[:], [CATF, D], f32, "catw")
            pbT_c = cload(pbT_i[:], [D, 4], f32, "pbT")
            relw_c = [[cload(relw_i[(l * 2 + r) * D : (l * 2 + r + 1) * D, :], [P, D], f32, f"relw{l}{r}") for r in range(2)] for l in range(2)]
            rootw_c = [cload(rootw_i[l * D : (l + 1) * D, :], [P, D], f32, f"rootw{l}") for l in range(2)]
            rgbT_c = cload(rgbT_i[:], [D, 2], f32, "rgbT")
            m1w_c = cload(m1w_i[:], [D, D], f32, "m1w")
            m1bT_c = cload(m1bT_i[:], [D, 1], f32, "m1bT")
            m2w_c = cload(m2w_i[:], [D, 2], f32, "m2w")
            m2bT_c = cload(m2bT_i[:], [2, 1], f32, "m2bT")
            offs_c = cp.tile([P, ncalls], i32, name="offs")
            nc.sync.dma_start(out=offs_c[:], in_=offs_i[:])
            cods_c = cp.tile([P, ncalls], f32, name="cods")
            nc.sync.dma_start(out=cods_c[:], in_=cods_i[:])
            invs_c = cp.tile([P, ncalls], f32, name="invs")
            nc.sync.dma_start(out=invs_c[:], in_=invs_i[:])
            iota_c = cload(iota_i[:], [P, 2 * W], f32, "iota")
            ident_c = cload(ident_i[:], [P, P], f32, "ident")
            zeros_c = cp.tile([S_PAD - S, D], f32, name="zeros")
            nc.vector.memset(zeros_c[:], 0.0)

            # resident transposed activations
            x0T = cp.tile([P, S_PAD], f32, name="x0T")
            x1T = cp.tile([P, S_PAD], f32, name="x1T")
            x2T = cp.tile([P, S_PAD], f32, name="x2T")

            # internal DRAM
            ag_in = [dp.tile([S_PAD, D], f32, name=f"ag_in{l}") for l in range(2)]
            tbl = [dp.tile([NTBL, D], f32, name=f"tbl{l}", addr_space="Shared") for l in range(2)]

            # ---- Stage 1: projection -> x0T ----
            for (j0, w) in widths:
                acc = wp.tile([P, w], f32, tag="projacc", bufs=2)
                for mi, (data, wgts) in enumerate([(des_i, desw_c), (tw_i, tww_c)]):
                    ps = pp.tile([P, w], f32, tag="proj", bufs=2)
                    for k in range(6):
                        dT = wp.tile([P, w], bf16, tag="dT", bufs=3)
                        nc.sync.dma_start(
                            out=dT[:],
                            in_=data[j0 : j0 + w, k * P : (k + 1) * P],
                            transpose=True,
                        )
                        nc.tensor.matmul(
                            out=ps[:], lhsT=wgts[k][:], rhs=dT[:],
                            start=(k == 0), stop=(k == 5),
                        )
                    if mi == 0:
                        nc.scalar.activation(
                            out=acc[:], in_=ps[:], func=AF.Lrelu,
                            bias=pbT_c[:, mi : mi + 1], scale=1.0, alpha=0.01,
                        )
                    else:
                        t = wp.tile([P, w], f32, tag="projt", bufs=2)
                        nc.scalar.activation(
                            out=t[:], in_=ps[:], func=AF.Lrelu,
                            bias=pbT_c[:, mi : mi + 1], scale=1.0, alpha=0.01,
                        )
                        nc.vector.tensor_tensor(out=acc[:], in0=acc[:], in1=t[:], op=OP.add)
                for mi, (dataT, wgt, nk) in enumerate(
                    [(numT_i, numw_c, NUMF), (catT_i, catw_c, CATF)]
                ):
                    ps = pp.tile([P, w], f32, tag="proj", bufs=2)
                    rT = wp.tile([nk, w], f32, tag=f"rT{mi}", bufs=2)
                    nc.sync.dma_start(out=rT[:], in_=dataT[:, j0 : j0 + w])
                    nc.tensor.matmul(out=ps[:], lhsT=wgt[:nk, :], rhs=rT[:], start=True, stop=True)
                    t = wp.tile([P, w], f32, tag="projt", bufs=2)
                    nc.scalar.activation(
                        out=t[:], in_=ps[:], func=AF.Lrelu,
                        bias=pbT_c[:, 2 + mi : 3 + mi], scale=1.0, alpha=0.01,
                    )
                    nc.vector.tensor_tensor(out=acc[:], in0=acc[:], in1=t[:], op=OP.add)
                nc.vector.tensor_copy(out=x0T[:, j0 : j0 + w], in_=acc[:])

            # ---- helper: write xT -> node-major table slice + AllGather ----
            def export_and_allgather(xT, l):
                for t in range(49):
                    ps = pp.tile([P, P], f32, tag="tr", bufs=1)
                    nc.tensor.transpose(
                        out=ps[:], in_=xT[:, t * P : (t + 1) * P], identity=ident_c[:]
                    )
                    xx = wp.tile([P, P], f32, tag="xrow", bufs=2)
                    nc.scalar.activation(out=xx[:], in_=ps[:], func=AF.Copy)
                    if t < 48:
                        nc.sync.dma_start(out=ag_in[l][t * P : (t + 1) * P, :], in_=xx[:])
                    else:
                        nc.sync.dma_start(out=ag_in[l][48 * P : S, :], in_=xx[: S - 48 * P, :])
                nc.sync.dma_start(out=ag_in[l][S:S_PAD, :], in_=zeros_c[:])
                nc.gpsimd.collective_compute(
                    "AllGather",
                    mybir.AluOpType.bypass,
                    replica_groups=[list(range(NC))],
                    ins=[ag_in[l].opt()],
                    outs=[tbl[l].opt()],
                )

            # ---- RGCN layer ----
            def layer(l, xT_in, xT_out):
                gk = 0
                for w in range(NWIN):
                    nch = chunks[w]
                    if nch:
                        agg_ps = pp.tile([P, 2 * W], f32, tag="agg", bufs=2)
                        for k in range(nch):
                            g = wp.tile([P, D], f32, tag="g", bufs=12)
                            nc.gpsimd.indirect_dma_start(
                                out=g[:],
                                out_offset=None,
                                in_=tbl[l][:],
                                in_offset=bass.IndirectOffsetOnAxis(
                                    ap=offs_c[:, gk : gk + 1], axis=0
                                ),
                            )
                            B = wp.tile([P, 2 * W], f32, tag="B", bufs=12)
                            nc.vector.tensor_scalar(
                                out=B[:], in0=iota_c[:],
                                scalar1=cods_c[:, gk : gk + 1],
                                scalar2=invs_c[:, gk : gk + 1],
                                op0=OP.is_equal, op1=OP.mult,
                            )
                            nc.tensor.matmul(
                                out=agg_ps[:], lhsT=g[:], rhs=B[:],
                                start=(k == 0), stop=(k == nch - 1),
                            )
                            gk += 1
                        aggT = wp.tile([P, 2 * W], f32, tag="aggs", bufs=2)
                        nc.vector.tensor_copy(out=aggT[:], in_=agg_ps[:])
                    out_ps = pp.tile([P, W], f32, tag="outw", bufs=2)
                    nc.tensor.matmul(
                        out=out_ps[:], lhsT=rootw_c[l][:],
                        rhs=xT_in[:, w * W : (w + 1) * W],
                        start=True, stop=(nch == 0),
                    )
                    if nch:
                        nc.tensor.matmul(
                            out=out_ps[:], lhsT=relw_c[l][0][:], rhs=aggT[:, 0:W],
                            start=False, stop=False,
                        )
                        nc.tensor.matmul(
                            out=out_ps[:], lhsT=relw_c[l][1][:], rhs=aggT[:, W : 2 * W],
                            start=False, stop=True,
                        )
                    nc.scalar.activation(
                        out=xT_out[:, w * W : (w + 1) * W], in_=out_ps[:],
                        func=AF.Lrelu, bias=rgbT_c[:, l : l + 1], scale=1.0, alpha=0.01,
                    )

            export_and_allgather(x0T, 0)
            layer(0, x0T, x1T)
            export_and_allgather(x1T, 1)
            layer(1, x1T, x2T)

            # ---- MLP head ----
            for (j0, w) in widths:
                ps = pp.tile([P, w], f32, tag="proj", bufs=2)
                nc.tensor.matmul(out=ps[:], lhsT=m1w_c[:], rhs=x2T[:, j0 : j0 + w], start=True, stop=True)
                hT = wp.tile([P, w], f32, tag="hT", bufs=2)
                nc.scalar.activation(out=hT[:], in_=ps[:], func=AF.Relu, bias=m1bT_c[:, 0:1])
                ps2 = pp.tile([2, w], f32, tag="mlp2", bufs=1)
                nc.tensor.matmul(out=ps2[:], lhsT=m2w_c[:], rhs=hT[:], start=True, stop=True)
                lg = wp.tile([2, w], f32, tag="lg", bufs=2)
                nc.vector.tensor_scalar(
                    out=lg[:], in0=ps2[:], scalar1=m2bT_c[:, 0:1], scalar2=None, op0=OP.add
                )
                nc.sync.dma_start(out=logitsT_o[:, j0 : j0 + w], in_=lg[:])

    nc.compile()
    return nc


def kernel(**inputs):
    global LAST_EXEC_NS
    import ml_dtypes
    from concourse import bass_utils

    edge_index = np.asarray(inputs["edge_index"])
    edge_type = np.asarray(inputs["edge_type"])
    des = np.asarray(inputs["des"], dtype=np.float32)
    tweets = np.asarray(inputs["tweets"], dtype=np.float32)
    num = np.asarray(inputs["num"], dtype=np.float32)
    cat = np.asarray(inputs["cat"], dtype=np.float32)

    chunks, per_core = _preprocess(edge_index, edge_type)

    key = (chunks, "v1")
    if key not in _CACHE:
        _CACHE[key] = _build_program(chunks)
    nc = _CACHE[key]

    bf16 = ml_dtypes.bfloat16
    f32 = np.float32

    def padrows(a, rows):
        out = np.zeros((rows, a.shape[1]), dtype=a.dtype)
        out[: a.shape[0]] = a
        return out

    relw = np.asarray(inputs["rel_w"], dtype=f32)      # [2,2,D,D]
    rootw = np.asarray(inputs["root_w"], dtype=f32)    # [2,D,D]
    rgb = np.asarray(inputs["rgcn_b"], dtype=f32)      # [2,D]
    pbT = np.stack(
        [np.asarray(inputs[k], dtype=f32) for k in ("des_b", "tweet_b", "num_b", "cat_b")],
        axis=1,
    )  # [D, 4]
    iota = np.broadcast_to(np.arange(2 * W, dtype=f32), (P, 2 * W)).copy()
    ident = np.eye(P, dtype=f32)

    common = dict(
        desw_i=np.asarray(inputs["des_w"], dtype=f32).astype(bf16),
        tww_i=np.asarray(inputs["tweet_w"], dtype=f32).astype(bf16),
        numw_i=np.asarray(inputs["num_w"], dtype=f32),
        catw_i=np.asarray(inputs["cat_w"], dtype=f32),
        pbT_i=pbT,
        relw_i=relw.reshape(4 * D, D),
        rootw_i=rootw.reshape(2 * D, D),
        rgbT_i=rgb.T.copy(),
        m1w_i=np.asarray(inputs["mlp_w1"], dtype=f32),
        m1bT_i=np.asarray(inputs["mlp_b1"], dtype=f32).reshape(D, 1),
        m2w_i=np.asarray(inputs["mlp_w2"], dtype=f32),
        m2bT_i=np.asarray(inputs["mlp_b2"], dtype=f32).reshape(2, 1),
        iota_i=iota,
        ident_i=ident,
    )

    in_maps = []
    for c in range(NC):
        sl = slice(c * S, (c + 1) * S)
        pc = per_core[c]
        in_maps.append(
            dict(
                common,
                des_i=padrows(des[sl].astype(bf16), S_PAD),
                tw_i=padrows(tweets[sl].astype(bf16), S_PAD),
                numT_i=np.ascontiguousarray(padrows(num[sl], S_PAD).T),
                catT_i=np.ascontiguousarray(padrows(cat[sl], S_PAD).T),
                offs_i=pc["offs"],
                cods_i=pc["cods"],
                invs_i=pc["invs"],
            )
        )

    trace = os.environ.get("KERNEL_TRACE", "0") == "1"
    res = bass_utils.run_bass_kernel_spmd(
        nc, in_maps, core_ids=list(range(NC)), trace=trace
    )
    LAST_EXEC_NS = res.exec_time_ns

    out = np.empty((N, 2), dtype=np.float32)
    for c in range(NC):
        lt = res.results[c]["logitsT_o"]  # [2, S_PAD]
        out[c * S : (c + 1) * S] = lt[:, :S].T
    return out


# revision 12
# speedup vs baseline: 1.0095x; 1.0095x over previous
"""BotRGCN Trainium2 kernel: 8-core SPMD bass kernel.

Strategy (graph/data parallel, per sharding hint):
- Nodes sharded: core c owns dst nodes [c*6250, (c+1)*6250).
- Multi-modal projection: each core computes x0 for its own nodes
  (des/tweets in bf16 via DMA-transpose + matmul, num/cat fp32).
- AllGather x across the 8 cores -> full node-feature table in DRAM.
- RGCN layer: edges (partitioned by dst) sorted by 32-dst windows;
  per 128-edge chunk: indirect-DMA gather of source rows, one-hot
  (iota==code) B matrix on DVE, TensorE matmul G^T.B accumulating
  aggT[fi, 64] per window (both relations side by side), scaled by
  host-precomputed 1/deg, then per-window epilogue matmuls
  (W_r^T @ aggT_r + root^T @ x_ownT) + leaky-relu -> x_{l+1}^T.
- MLP head on each core's own slice; host concatenates [N,2] output.
"""

import os
import numpy as np

NC = 8
N = 50000
S = 6250          # dst nodes per core
S_PAD = 6272      # = 49*128
NTBL = NC * S_PAD  # 50176
D = 128
W = 32            # dsts per aggregation window
NWIN = S_PAD // W  # 196
E = 1600000
DES = 768
TWE = 768
NUMF = 5
CATF = 6
P = 128
ZERO_ROW = S      # row 6250 of slice 0 is a zero pad row (global row index S)

LAST_EXEC_NS = None

_CACHE = {}


def _preprocess(edge_index, edge_type):
    """Sort/pad edges per core into fixed chunk structure shared by all cores.

    Returns (chunks, per_core) where chunks[w] = number of 128-edge chunks for
    window w (same for every core = max over cores), and per_core is a list of
    dicts with offs [128, ncalls] i32, codes [128, ncalls] f32,
    invc [1, NWIN*2*W] f32.
    """
    src = np.asarray(edge_index[0], dtype=np.int64)
    dst = np.asarray(edge_index[1], dtype=np.int64)
    et = np.asarray(edge_type, dtype=np.int64)

    core = dst // S
    dloc = dst - core * S
    win = dloc // W
    code = (dloc - win * W) + W * et          # 0..63
    gsrc = (src // S) * S_PAD + (src % S)     # global table row of src

    per_core_data = []
    cnts = np.zeros((NC, NWIN), dtype=np.int64)
    for c in range(NC):
        m = core == c
        w_c = win[m]
        g_c = gsrc[m]
        cd_c = code[m]
        d_c = dloc[m]
        r_c = et[m]
        order = np.lexsort((g_c, w_c))
        w_c, g_c, cd_c = w_c[order], g_c[order], cd_c[order]
        d_c, r_c = d_c[order], r_c[order]
        cnt = np.bincount(w_c, minlength=NWIN)
        cnts[c] = cnt
        # per (dloc, rel) in-degree for mean
        deg = np.bincount(d_c * 2 + r_c, minlength=S * 2).reshape(S, 2)
        per_core_data.append((g_c, cd_c, d_c, r_c, cnt, deg))

    chunks = ((cnts.max(axis=0) + 127) // 128).astype(np.int64)
    ncalls = int(chunks.sum())
    starts = np.concatenate([[0], np.cumsum(chunks)])

    per_core = []
    for c in range(NC):
        g_c, cd_c, d_c, r_c, cnt, deg = per_core_data[c]
        inv_e = (1.0 / np.maximum(deg[d_c, r_c], 1.0)).astype(np.float32)
        offs = np.full(ncalls * P, ZERO_ROW, dtype=np.int32)
        cods = np.zeros(ncalls * P, dtype=np.float32)
        invs = np.zeros(ncalls * P, dtype=np.float32)
        epos = np.concatenate([[0], np.cumsum(cnt)])
        for w in range(NWIN):
            n = int(cnt[w])
            if n == 0:
                continue
            base = int(starts[w]) * P
            offs[base : base + n] = g_c[epos[w] : epos[w + 1]]
            cods[base : base + n] = cd_c[epos[w] : epos[w + 1]]
            invs[base : base + n] = inv_e[epos[w] : epos[w + 1]]
        # [ncalls*P] -> [P, ncalls]: call k partition p = element k*P+p
        offs = offs.reshape(ncalls, P).T.copy()
        cods = cods.reshape(ncalls, P).T.copy()
        invs = invs.reshape(ncalls, P).T.copy()
        per_core.append(dict(offs=offs, cods=cods, invs=invs))
    return tuple(int(x) for x in chunks), per_core


def _build_program(chunks):
    import concourse.bass as bass
    import concourse.bacc as bacc
    import concourse.mybir as mybir
    import concourse.tile as tile

    f32 = mybir.dt.float32
    bf16 = mybir.dt.bfloat16
    i32 = mybir.dt.int32
    AF = mybir.ActivationFunctionType
    OP = mybir.AluOpType

    ncalls = int(sum(chunks))

    nc = bacc.Bacc("TRN2", target_bir_lowering=False, debug=False, num_devices=NC)

    def din(name, shape, dt):
        return nc.dram_tensor(name, shape, dt, kind="ExternalInput").ap()

    des_i = din("des_i", [S_PAD, DES], bf16)
    tw_i = din("tw_i", [S_PAD, TWE], bf16)
    numT_i = din("numT_i", [NUMF, S_PAD], f32)
    catT_i = din("catT_i", [CATF, S_PAD], f32)
    desw_i = din("desw_i", [DES, D], bf16)
    tww_i = din("tww_i", [TWE, D], bf16)
    numw_i = din("numw_i", [NUMF, D], f32)
    catw_i = din("catw_i", [CATF, D], f32)
    pbT_i = din("pbT_i", [D, 4], f32)          # proj biases per modality
    relw_i = din("relw_i", [2 * 2 * D, D], bf16)  # (l,r) blocks of [fi,fo]
    rootw_i = din("rootw_i", [2 * D, D], bf16)
    rgbT_i = din("rgbT_i", [D, 2], f32)
    m1w_i = din("m1w_i", [D, D], bf16)
    m1bT_i = din("m1bT_i", [D, 1], f32)
    m2w_i = din("m2w_i", [D, 2], bf16)
    m2bT_i = din("m2bT_i", [2, 1], f32)
    offs_i = din("offs_i", [P, ncalls], i32)
    cods_i = din("cods_i", [P, ncalls], f32)
    invs_i = din("invs_i", [P, ncalls], f32)
    iota_i = din("iota_i", [P, 2 * W], bf16)
    ident_i = din("ident_i", [P, P], bf16)

    logitsT_o = nc.dram_tensor("logitsT_o", [2, S_PAD], f32, kind="ExternalOutput").ap()

    # node tile widths for projection / MLP
    widths = [(i * 512, 512) for i in range(12)] + [(12 * 512, 128)]

    with tile.TileContext(nc) as tc:
        with (
            tc.tile_pool(name="const", bufs=1) as cp,
            tc.tile_pool(name="work", bufs=2) as wp,
            tc.tile_pool(name="psum", bufs=1, space="PSUM") as pp,
            tc.tile_pool(name="dram", bufs=1, space="DRAM") as dp,
        ):
            # ---- constants to SBUF ----
            def cload(ap_in, shape, dt, name):
                t = cp.tile(shape, dt, name=name)
                nc.sync.dma_start(out=t[:], in_=ap_in)
                return t

            desw_c = [cload(desw_i[k * P : (k + 1) * P, :], [P, D], bf16, f"desw{k}") for k in range(6)]
            tww_c = [cload(tww_i[k * P : (k + 1) * P, :], [P, D], bf16, f"tww{k}") for k in range(6)]
            numw_c = cload(numw_i[:], [NUMF, D], f32, "numw")
            catw_c = cload(catw_i[:], [CATF, D], f32, "catw")
            pbT_c = cload(pbT_i[:], [D, 4], f32, "pbT")
            relw_c = [[cload(relw_i[(l * 2 + r) * D : (l * 2 + r + 1) * D, :], [P, D], bf16, f"relw{l}{r}") for r in range(2)] for l in range(2)]
            rootw_c = [cload(rootw_i[l * D : (l + 1) * D, :], [P, D], bf16, f"rootw{l}") for l in range(2)]
            rgbT_c = cload(rgbT_i[:], [D, 2], f32, "rgbT")
            m1w_c = cload(m1w_i[:], [D, D], bf16, "m1w")
            m1bT_c = cload(m1bT_i[:], [D, 1], f32, "m1bT")
            m2w_c = cload(m2w_i[:], [D, 2], bf16, "m2w")
            m2bT_c = cload(m2bT_i[:], [2, 1], f32, "m2bT")
            offs_c = cp.tile([P, ncalls], i32, name="offs")
            nc.sync.dma_start(out=offs_c[:], in_=offs_i[:])
            cods_c = cp.tile([P, ncalls], f32, name="cods")
            nc.sync.dma_start(out=cods_c[:], in_=cods_i[:])
            invs_c = cp.tile([P, ncalls], f32, name="invs")
            nc.sync.dma_start(out=invs_c[:], in_=invs_i[:])
            iota_c = cload(iota_i[:], [P, 2 * W], bf16, "iota")
            ident_c = cload(ident_i[:], [P, P], bf16, "ident")
            zeros_c = cp.tile([S_PAD - S, D], bf16, name="zeros")
            nc.vector.memset(zeros_c[:], 0.0)

            # resident transposed activations
            x0T = cp.tile([P, S_PAD], bf16, name="x0T")
            x1T = cp.tile([P, S_PAD], bf16, name="x1T")
            x2T = cp.tile([P, S_PAD], bf16, name="x2T")

            # internal DRAM
            ag_in = [dp.tile([S_PAD, D], bf16, name=f"ag_in{l}") for l in range(2)]
            tbl = [dp.tile([NTBL, D], bf16, name=f"tbl{l}", addr_space="Shared") for l in range(2)]

            # ---- Stage 1: projection -> x0T ----
            for (j0, w) in widths:
                acc = wp.tile([P, w], f32, tag="projacc", bufs=2)
                for mi, (data, wgts) in enumerate([(des_i, desw_c), (tw_i, tww_c)]):
                    ps = pp.tile([P, w], f32, tag="proj", bufs=2)
                    for k in range(6):
                        dT = wp.tile([P, w], bf16, tag="dT", bufs=3)
                        nc.sync.dma_start(
                            out=dT[:],
                            in_=data[j0 : j0 + w, k * P : (k + 1) * P],
                            transpose=True,
                        )
                        nc.tensor.matmul(
                            out=ps[:], lhsT=wgts[k][:], rhs=dT[:],
                            start=(k == 0), stop=(k == 5),
                        )
                    if mi == 0:
                        nc.scalar.activation(
                            out=acc[:], in_=ps[:], func=AF.Lrelu,
                            bias=pbT_c[:, mi : mi + 1], scale=1.0, alpha=0.01,
                        )
                    else:
                        t = wp.tile([P, w], f32, tag="projt", bufs=2)
                        nc.scalar.activation(
                            out=t[:], in_=ps[:], func=AF.Lrelu,
                            bias=pbT_c[:, mi : mi + 1], scale=1.0, alpha=0.01,
                        )
                        nc.vector.tensor_tensor(out=acc[:], in0=acc[:], in1=t[:], op=OP.add)
                for mi, (dataT, wgt, nk) in enumerate(
                    [(numT_i, numw_c, NUMF), (catT_i, catw_c, CATF)]
                ):
                    ps = pp.tile([P, w], f32, tag="proj", bufs=2)
                    rT = wp.tile([nk, w], f32, tag=f"rT{mi}", bufs=2)
                    nc.sync.dma_start(out=rT[:], in_=dataT[:, j0 : j0 + w])
                    nc.tensor.matmul(out=ps[:], lhsT=wgt[:nk, :], rhs=rT[:], start=True, stop=True)
                    t = wp.tile([P, w], f32, tag="projt", bufs=2)
                    nc.scalar.activation(
                        out=t[:], in_=ps[:], func=AF.Lrelu,
                        bias=pbT_c[:, 2 + mi : 3 + mi], scale=1.0, alpha=0.01,
                    )
                    nc.vector.tensor_tensor(out=acc[:], in0=acc[:], in1=t[:], op=OP.add)
                nc.vector.tensor_copy(out=x0T[:, j0 : j0 + w], in_=acc[:])

            # ---- helper: write xT -> node-major table slice + AllGather ----
            def export_and_allgather(xT, l):
                for t in range(49):
                    ps = pp.tile([P, P], bf16, tag="tr", bufs=1)
                    nc.tensor.transpose(
                        out=ps[:], in_=xT[:, t * P : (t + 1) * P], identity=ident_c[:]
                    )
                    xx = wp.tile([P, P], bf16, tag="xrow", bufs=2)
                    nc.scalar.activation(out=xx[:], in_=ps[:], func=AF.Copy)
                    if t < 48:
                        nc.sync.dma_start(out=ag_in[l][t * P : (t + 1) * P, :], in_=xx[:])
                    else:
                        nc.sync.dma_start(out=ag_in[l][48 * P : S, :], in_=xx[: S - 48 * P, :])
                nc.sync.dma_start(out=ag_in[l][S:S_PAD, :], in_=zeros_c[:])
                nc.gpsimd.collective_compute(
                    "AllGather",
                    mybir.AluOpType.bypass,
                    replica_groups=[list(range(NC))],
                    ins=[ag_in[l].opt()],
                    outs=[tbl[l].opt()],
                )

            # ---- RGCN layer ----
            def layer(l, xT_in, xT_out):
                gk = 0
                for w in range(NWIN):
                    nch = chunks[w]
                    if nch:
                        agg_ps = pp.tile([P, 2 * W], f32, tag="agg", bufs=2)
                        for k in range(nch):
                            g = wp.tile([P, D], bf16, tag="g", bufs=16)
                            nc.gpsimd.indirect_dma_start(
                                out=g[:],
                                out_offset=None,
                                in_=tbl[l][:],
                                in_offset=bass.IndirectOffsetOnAxis(
                                    ap=offs_c[:, gk : gk + 1], axis=0
                                ),
                            )
                            B = wp.tile([P, 2 * W], bf16, tag="B", bufs=16)
                            nc.vector.tensor_scalar(
                                out=B[:], in0=iota_c[:],
                                scalar1=cods_c[:, gk : gk + 1],
                                scalar2=invs_c[:, gk : gk + 1],
                                op0=OP.is_equal, op1=OP.mult,
                            )
                            nc.tensor.matmul(
                                out=agg_ps[:], lhsT=g[:], rhs=B[:],
                                start=(k == 0), stop=(k == nch - 1),
                            )
                            gk += 1
                        aggT = wp.tile([P, 2 * W], bf16, tag="aggs", bufs=2)
                        nc.vector.tensor_copy(out=aggT[:], in_=agg_ps[:])
                    out_ps = pp.tile([P, W], f32, tag="outw", bufs=2)
                    nc.tensor.matmul(
                        out=out_ps[:], lhsT=rootw_c[l][:],
                        rhs=xT_in[:, w * W : (w + 1) * W],
                        start=True, stop=(nch == 0),
                    )
                    if nch:
                        nc.tensor.matmul(
                            out=out_ps[:], lhsT=relw_c[l][0][:], rhs=aggT[:, 0:W],
                            start=False, stop=False,
                        )
                        nc.tensor.matmul(
                            out=out_ps[:], lhsT=relw_c[l][1][:], rhs=aggT[:, W : 2 * W],
                            start=False, stop=True,
                        )
                    nc.scalar.activation(
                        out=xT_out[:, w * W : (w + 1) * W], in_=out_ps[:],
                        func=AF.Lrelu, bias=rgbT_c[:, l : l + 1], scale=1.0, alpha=0.01,
                    )

            export_and_allgather(x0T, 0)
            layer(0, x0T, x1T)
            export_and_allgather(x1T, 1)
            layer(1, x1T, x2T)

            # ---- MLP head ----
            for (j0, w) in widths:
                ps = pp.tile([P, w], f32, tag="proj", bufs=2)
                nc.tensor.matmul(out=ps[:], lhsT=m1w_c[:], rhs=x2T[:, j0 : j0 + w], start=True, stop=True)
                hT = wp.tile([P, w], bf16, tag="hT", bufs=2)
                nc.scalar.activation(out=hT[:], in_=ps[:], func=AF.Relu, bias=m1bT_c[:, 0:1])
                ps2 = pp.tile([2, w], f32, tag="mlp2", bufs=1)
                nc.tensor.matmul(out=ps2[:], lhsT=m2w_c[:], rhs=hT[:], start=True, stop=True)
                lg = wp.tile([2, w], f32, tag="lg", bufs=2)
                nc.vector.tensor_scalar(
                    out=lg[:], in0=ps2[:], scalar1=m2bT_c[:, 0:1], scalar2=None, op0=OP.add
                )
                nc.sync.dma_start(out=logitsT_o[:, j0 : j0 + w], in_=lg[:])

    nc.compile()
    return nc


def kernel(**inputs):
    global LAST_EXEC_NS
    import ml_dtypes
    from concourse import bass_utils

    edge_index = np.asarray(inputs["edge_index"])
    edge_type = np.asarray(inputs["edge_type"])
    des = np.asarray(inputs["des"], dtype=np.float32)
    tweets = np.asarray(inputs["tweets"], dtype=np.float32)
    num = np.asarray(inputs["num"], dtype=np.float32)
    cat = np.asarray(inputs["cat"], dtype=np.float32)

    chunks, per_core = _preprocess(edge_index, edge_type)

    key = (chunks, "v2")
    if key not in _CACHE:
        _CACHE[key] = _build_program(chunks)
    nc = _CACHE[key]

    bf16 = ml_dtypes.bfloat16
    f32 = np.float32

    def padrows(a, rows):
        out = np.zeros((rows, a.shape[1]), dtype=a.dtype)
        out[: a.shape[0]] = a
        return out

    relw = np.asarray(inputs["rel_w"], dtype=f32)      # [2,2,D,D]
    rootw = np.asarray(inputs["root_w"], dtype=f32)    # [2,D,D]
    rgb = np.asarray(inputs["rgcn_b"], dtype=f32)      # [2,D]
    pbT = np.stack(
        [np.asarray(inputs[k], dtype=f32) for k in ("des_b", "tweet_b", "num_b", "cat_b")],
        axis=1,
    )  # [D, 4]
    iota = np.broadcast_to(np.arange(2 * W, dtype=f32), (P, 2 * W)).copy()
    ident = np.eye(P, dtype=f32)

    common = dict(
        desw_i=np.asarray(inputs["des_w"], dtype=f32).astype(bf16),
        tww_i=np.asarray(inputs["tweet_w"], dtype=f32).astype(bf16),
        numw_i=np.asarray(inputs["num_w"], dtype=f32),
        catw_i=np.asarray(inputs["cat_w"], dtype=f32),
        pbT_i=pbT,
        relw_i=relw.reshape(4 * D, D).astype(bf16),
        rootw_i=rootw.reshape(2 * D, D).astype(bf16),
        rgbT_i=rgb.T.copy(),
        m1w_i=np.asarray(inputs["mlp_w1"], dtype=f32).astype(bf16),
        m1bT_i=np.asarray(inputs["mlp_b1"], dtype=f32).reshape(D, 1),
        m2w_i=np.asarray(inputs["mlp_w2"], dtype=f32).astype(bf16),
        m2bT_i=np.asarray(inputs["mlp_b2"], dtype=f32).reshape(2, 1),
        iota_i=iota.astype(bf16),
        ident_i=ident.astype(bf16),
    )

    in_maps = []
    for c in range(NC):
        sl = slice(c * S, (c + 1) * S)
        pc = per_core[c]
        in_maps.append(
            dict(
                common,
                des_i=padrows(des[sl].astype(bf16), S_PAD),
                tw_i=padrows(tweets[sl].astype(bf16), S_PAD),
                numT_i=np.ascontiguousarray(padrows(num[sl], S_PAD).T),
                catT_i=np.ascontiguousarray(padrows(cat[sl], S_PAD).T),
                offs_i=pc["offs"],
                cods_i=pc["cods"],
                invs_i=pc["invs"],
            )
        )

    trace = os.environ.get("KERNEL_TRACE", "0") == "1"
    res = bass_utils.run_bass_kernel_spmd(
        nc, in_maps, core_ids=list(range(NC)), trace=trace
    )
    LAST_EXEC_NS = res.exec_time_ns

    out = np.empty((N, 2), dtype=np.float32)
    for c in range(NC):
        lt = res.results[c]["logitsT_o"]  # [2, S_PAD]
        out[c * S : (c + 1) * S] = lt[:, :S].T
    return out


# revision 13
# speedup vs baseline: 1.0227x; 1.0130x over previous
"""BotRGCN Trainium2 kernel: 8-core SPMD bass kernel.

Strategy (graph/data parallel, per sharding hint):
- Nodes sharded: core c owns dst nodes [c*6250, (c+1)*6250).
- Multi-modal projection: each core computes x0 for its own nodes
  (des/tweets in bf16 via DMA-transpose + matmul, num/cat fp32).
- AllGather x across the 8 cores -> full node-feature table in DRAM.
- RGCN layer: edges (partitioned by dst) sorted by 32-dst windows;
  per 128-edge chunk: indirect-DMA gather of source rows, one-hot
  (iota==code) B matrix on DVE, TensorE matmul G^T.B accumulating
  aggT[fi, 64] per window (both relations side by side), scaled by
  host-precomputed 1/deg, then per-window epilogue matmuls
  (W_r^T @ aggT_r + root^T @ x_ownT) + leaky-relu -> x_{l+1}^T.
- MLP head on each core's own slice; host concatenates [N,2] output.
"""

import os
import numpy as np

NC = 8
N = 50000
S = 6250          # dst nodes per core
S_PAD = 6272      # = 49*128
NTBL = NC * S_PAD  # 50176
D = 128
W = 32            # dsts per aggregation window
NWIN = S_PAD // W  # 196
E = 1600000
DES = 768
TWE = 768
NUMF = 5
CATF = 6
P = 128
ZERO_ROW = S      # row 6250 of slice 0 is a zero pad row (global row index S)

LAST_EXEC_NS = None

_CACHE = {}


def _preprocess(edge_index, edge_type):
    """Sort/pad edges per core into fixed chunk structure shared by all cores.

    Returns (chunks, per_core) where chunks[w] = number of 128-edge chunks for
    window w (same for every core = max over cores), and per_core is a list of
    dicts with offs [128, ncalls] i32, codes [128, ncalls] f32,
    invc [1, NWIN*2*W] f32.
    """
    src = np.asarray(edge_index[0], dtype=np.int64)
    dst = np.asarray(edge_index[1], dtype=np.int64)
    et = np.asarray(edge_type, dtype=np.int64)

    core = dst // S
    dloc = dst - core * S
    win = dloc // W
    code = (dloc - win * W) + W * et          # 0..63
    gsrc = (src // S) * S_PAD + (src % S)     # global table row of src

    per_core_data = []
    cnts = np.zeros((NC, NWIN), dtype=np.int64)
    for c in range(NC):
        m = core == c
        w_c = win[m]
        g_c = gsrc[m]
        cd_c = code[m]
        d_c = dloc[m]
        r_c = et[m]
        order = np.lexsort((g_c, w_c))
        w_c, g_c, cd_c = w_c[order], g_c[order], cd_c[order]
        d_c, r_c = d_c[order], r_c[order]
        cnt = np.bincount(w_c, minlength=NWIN)
        cnts[c] = cnt
        # per (dloc, rel) in-degree for mean
        deg = np.bincount(d_c * 2 + r_c, minlength=S * 2).reshape(S, 2)
        per_core_data.append((g_c, cd_c, d_c, r_c, cnt, deg))

    chunks = ((cnts.max(axis=0) + 127) // 128).astype(np.int64)
    ncalls = int(chunks.sum())
    starts = np.concatenate([[0], np.cumsum(chunks)])

    per_core = []
    for c in range(NC):
        g_c, cd_c, d_c, r_c, cnt, deg = per_core_data[c]
        inv_e = (1.0 / np.maximum(deg[d_c, r_c], 1.0)).astype(np.float32)
        offs = np.full(ncalls * P, ZERO_ROW, dtype=np.int32)
        cods = np.zeros(ncalls * P, dtype=np.float32)
        invs = np.zeros(ncalls * P, dtype=np.float32)
        epos = np.concatenate([[0], np.cumsum(cnt)])
        for w in range(NWIN):
            n = int(cnt[w])
            if n == 0:
                continue
            base = int(starts[w]) * P
            offs[base : base + n] = g_c[epos[w] : epos[w + 1]]
            cods[base : base + n] = cd_c[epos[w] : epos[w + 1]]
            invs[base : base + n] = inv_e[epos[w] : epos[w + 1]]
        # [ncalls*P] -> [P, ncalls]: call k partition p = element k*P+p
        offs = offs.reshape(ncalls, P).T.copy()
        cods = cods.reshape(ncalls, P).T.copy()
        invs = invs.reshape(ncalls, P).T.copy()
        per_core.append(dict(offs=offs, cods=cods, invs=invs))
    return tuple(int(x) for x in chunks), per_core


def _build_program(chunks):
    import concourse.bass as bass
    import concourse.bacc as bacc
    import concourse.mybir as mybir
    import concourse.tile as tile

    f32 = mybir.dt.float32
    bf16 = mybir.dt.bfloat16
    i32 = mybir.dt.int32
    AF = mybir.ActivationFunctionType
    OP = mybir.AluOpType

    ncalls = int(sum(chunks))

    nc = bacc.Bacc("TRN2", target_bir_lowering=False, debug=False, num_devices=NC)

    def din(name, shape, dt):
        return nc.dram_tensor(name, shape, dt, kind="ExternalInput").ap()

    desT_i = din("desT_i", [DES, S_PAD], bf16)
    twT_i = din("twT_i", [TWE, S_PAD], bf16)
    numT_i = din("numT_i", [NUMF, S_PAD], f32)
    catT_i = din("catT_i", [CATF, S_PAD], f32)
    desw_i = din("desw_i", [DES, D], bf16)
    tww_i = din("tww_i", [TWE, D], bf16)
    numw_i = din("numw_i", [NUMF, D], f32)
    catw_i = din("catw_i", [CATF, D], f32)
    pbT_i = din("pbT_i", [D, 4], f32)          # proj biases per modality
    relw_i = din("relw_i", [2 * 2 * D, D], f32)  # (l,r) blocks of [fi,fo]
    rootw_i = din("rootw_i", [2 * D, D], f32)
    rgbT_i = din("rgbT_i", [D, 2], f32)
    m1w_i = din("m1w_i", [D, D], f32)
    m1bT_i = din("m1bT_i", [D, 1], f32)
    m2w_i = din("m2w_i", [D, 2], f32)
    m2bT_i = din("m2bT_i", [2, 1], f32)
    offs_i = din("offs_i", [P, ncalls], i32)
    cods_i = din("cods_i", [P, ncalls], f32)
    invs_i = din("invs_i", [P, ncalls], f32)
    iota_i = din("iota_i", [P, 2 * W], f32)
    ident_i = din("ident_i", [P, P], f32)

    logitsT_o = nc.dram_tensor("logitsT_o", [2, S_PAD], f32, kind="ExternalOutput").ap()

    # node tile widths for projection / MLP
    widths = [(i * 512, 512) for i in range(12)] + [(12 * 512, 128)]

    with tile.TileContext(nc) as tc:
        with (
            tc.tile_pool(name="const", bufs=1) as cp,
            tc.tile_pool(name="work", bufs=2) as wp,
            tc.tile_pool(name="psum", bufs=1, space="PSUM") as pp,
            tc.tile_pool(name="dram", bufs=1, space="DRAM") as dp,
        ):
            # ---- constants to SBUF ----
            def cload(ap_in, shape, dt, name):
                t = cp.tile(shape, dt, name=name)
                nc.sync.dma_start(out=t[:], in_=ap_in)
                return t

            desw_c = [cload(desw_i[k * P : (k + 1) * P, :], [P, D], bf16, f"desw{k}") for k in range(6)]
            tww_c = [cload(tww_i[k * P : (k + 1) * P, :], [P, D], bf16, f"tww{k}") for k in range(6)]
            numw_c = cload(numw_i[:], [NUMF, D], f32, "numw")
            catw_c = cload(catw_i[:], [CATF, D], f32, "catw")
            pbT_c = cload(pbT_i[:], [D, 4], f32, "pbT")
            relw_c = [[cload(relw_i[(l * 2 + r) * D : (l * 2 + r + 1) * D, :], [P, D], f32, f"relw{l}{r}") for r in range(2)] for l in range(2)]
            rootw_c = [cload(rootw_i[l * D : (l + 1) * D, :], [P, D], f32, f"rootw{l}") for l in range(2)]
            rgbT_c = cload(rgbT_i[:], [D, 2], f32, "rgbT")
            m1w_c = cload(m1w_i[:], [D, D], f32, "m1w")
            m1bT_c = cload(m1bT_i[:], [D, 1], f32, "m1bT")
            m2w_c = cload(m2w_i[:], [D, 2], f32, "m2w")
            m2bT_c = cload(m2bT_i[:], [2, 1], f32, "m2bT")
            offs_c = cp.tile([P, ncalls], i32, name="offs")
            nc.sync.dma_start(out=offs_c[:], in_=offs_i[:])
            cods_c = cp.tile([P, ncalls], f32, name="cods")
            nc.sync.dma_start(out=cods_c[:], in_=cods_i[:])
            invs_c = cp.tile([P, ncalls], f32, name="invs")
            nc.sync.dma_start(out=invs_c[:], in_=invs_i[:])
            iota_c = cload(iota_i[:], [P, 2 * W], f32, "iota")
            ident_c = cload(ident_i[:], [P, P], f32, "ident")
            zeros_c = cp.tile([S_PAD - S, D], f32, name="zeros")
            nc.vector.memset(zeros_c[:], 0.0)

            # resident transposed activations
            x0T = cp.tile([P, S_PAD], f32, name="x0T")
            x1T = cp.tile([P, S_PAD], f32, name="x1T")
            x2T = cp.tile([P, S_PAD], f32, name="x2T")

            # internal DRAM
            ag_in = [dp.tile([S_PAD, D], f32, name=f"ag_in{l}") for l in range(2)]
            tbl = [dp.tile([NTBL, D], f32, name=f"tbl{l}", addr_space="Shared") for l in range(2)]

            # ---- Stage 1: projection -> x0T ----
            for (j0, w) in widths:
                acc = wp.tile([P, w], f32, tag="projacc", bufs=2)
                for mi, (dataT, wgts) in enumerate([(desT_i, desw_c), (twT_i, tww_c)]):
                    ps = pp.tile([P, w], f32, tag="proj", bufs=2)
                    for k in range(6):
                        dT = wp.tile([P, w], bf16, tag="dT", bufs=3)
                        nc.sync.dma_start(
                            out=dT[:],
                            in_=dataT[k * P : (k + 1) * P, j0 : j0 + w],
                        )
                        nc.tensor.matmul(
                            out=ps[:], lhsT=wgts[k][:], rhs=dT[:],
                            start=(k == 0), stop=(k == 5),
                        )
                    if mi == 0:
                        nc.scalar.activation(
                            out=acc[:], in_=ps[:], func=AF.Lrelu,
                            bias=pbT_c[:, mi : mi + 1], scale=1.0, alpha=0.01,
                        )
                    else:
                        t = wp.tile([P, w], f32, tag="projt", bufs=2)
                        nc.scalar.activation(
                            out=t[:], in_=ps[:], func=AF.Lrelu,
                            bias=pbT_c[:, mi : mi + 1], scale=1.0, alpha=0.01,
                        )
                        nc.vector.tensor_tensor(out=acc[:], in0=acc[:], in1=t[:], op=OP.add)
                for mi, (dataT, wgt, nk) in enumerate(
                    [(numT_i, numw_c, NUMF), (catT_i, catw_c, CATF)]
                ):
                    ps = pp.tile([P, w], f32, tag="proj", bufs=2)
                    rT = wp.tile([nk, w], f32, tag=f"rT{mi}", bufs=2)
                    nc.sync.dma_start(out=rT[:], in_=dataT[:, j0 : j0 + w])
                    nc.tensor.matmul(out=ps[:], lhsT=wgt[:nk, :], rhs=rT[:], start=True, stop=True)
                    t = wp.tile([P, w], f32, tag="projt", bufs=2)
                    nc.scalar.activation(
                        out=t[:], in_=ps[:], func=AF.Lrelu,
                        bias=pbT_c[:, 2 + mi : 3 + mi], scale=1.0, alpha=0.01,
                    )
                    nc.vector.tensor_tensor(out=acc[:], in0=acc[:], in1=t[:], op=OP.add)
                nc.vector.tensor_copy(out=x0T[:, j0 : j0 + w], in_=acc[:])

            # ---- helper: write xT -> node-major table slice + AllGather ----
            def export_and_allgather(xT, l):
                for t in range(49):
                    ps = pp.tile([P, P], f32, tag="tr", bufs=1)
                    nc.tensor.transpose(
                        out=ps[:], in_=xT[:, t * P : (t + 1) * P], identity=ident_c[:]
                    )
                    xx = wp.tile([P, P], f32, tag="xrow", bufs=2)
                    nc.scalar.activation(out=xx[:], in_=ps[:], func=AF.Copy)
                    if t < 48:
                        nc.sync.dma_start(out=ag_in[l][t * P : (t + 1) * P, :], in_=xx[:])
                    else:
                        nc.sync.dma_start(out=ag_in[l][48 * P : S, :], in_=xx[: S - 48 * P, :])
                nc.sync.dma_start(out=ag_in[l][S:S_PAD, :], in_=zeros_c[:])
                nc.gpsimd.collective_compute(
                    "AllGather",
                    mybir.AluOpType.bypass,
                    replica_groups=[list(range(NC))],
                    ins=[ag_in[l].opt()],
                    outs=[tbl[l].opt()],
                )

            # ---- RGCN layer ----
            def layer(l, xT_in, xT_out):
                gk = 0
                for w in range(NWIN):
                    nch = chunks[w]
                    if nch:
                        agg_ps = pp.tile([P, 2 * W], f32, tag="agg", bufs=2)
                        for k in range(nch):
                            g = wp.tile([P, D], f32, tag="g", bufs=16)
                            nc.gpsimd.indirect_dma_start(
                                out=g[:],
                                out_offset=None,
                                in_=tbl[l][:],
                                in_offset=bass.IndirectOffsetOnAxis(
                                    ap=offs_c[:, gk : gk + 1], axis=0
                                ),
                            )
                            B = wp.tile([P, 2 * W], f32, tag="B", bufs=16)
                            nc.vector.tensor_scalar(
                                out=B[:], in0=iota_c[:],
                                scalar1=cods_c[:, gk : gk + 1],
                                scalar2=invs_c[:, gk : gk + 1],
                                op0=OP.is_equal, op1=OP.mult,
                            )
                            nc.tensor.matmul(
                                out=agg_ps[:], lhsT=g[:], rhs=B[:],
                                start=(k == 0), stop=(k == nch - 1),
                            )
                            gk += 1
                        aggT = wp.tile([P, 2 * W], f32, tag="aggs", bufs=2)
                        nc.vector.tensor_copy(out=aggT[:], in_=agg_ps[:])
                    out_ps = pp.tile([P, W], f32, tag="outw", bufs=2)
                    nc.tensor.matmul(
                        out=out_ps[:], lhsT=rootw_c[l][:],
                        rhs=xT_in[:, w * W : (w + 1) * W],
                        start=True, stop=(nch == 0),
                    )
                    if nch:
                        nc.tensor.matmul(
                            out=out_ps[:], lhsT=relw_c[l][0][:], rhs=aggT[:, 0:W],
                            start=False, stop=False,
                        )
                        nc.tensor.matmul(
                            out=out_ps[:], lhsT=relw_c[l][1][:], rhs=aggT[:, W : 2 * W],
                            start=False, stop=True,
                        )
                    nc.scalar.activation(
                        out=xT_out[:, w * W : (w + 1) * W], in_=out_ps[:],
                        func=AF.Lrelu, bias=rgbT_c[:, l : l + 1], scale=1.0, alpha=0.01,
                    )

            export_and_allgather(x0T, 0)
            layer(0, x0T, x1T)
            export_and_allgather(x1T, 1)
            layer(1, x1T, x2T)

            # ---- MLP head ----
            for (j0, w) in widths:
                ps = pp.tile([P, w], f32, tag="proj", bufs=2)
                nc.tensor.matmul(out=ps[:], lhsT=m1w_c[:], rhs=x2T[:, j0 : j0 + w], start=True, stop=True)
                hT = wp.tile([P, w], f32, tag="hT", bufs=2)
                nc.scalar.activation(out=hT[:], in_=ps[:], func=AF.Relu, bias=m1bT_c[:, 0:1])
                ps2 = pp.tile([2, w], f32, tag="mlp2", bufs=1)
                nc.tensor.matmul(out=ps2[:], lhsT=m2w_c[:], rhs=hT[:], start=True, stop=True)
                lg = wp.tile([2, w], f32, tag="lg", bufs=2)
                nc.vector.tensor_scalar(
                    out=lg[:], in0=ps2[:], scalar1=m2bT_c[:, 0:1], scalar2=None, op0=OP.add
                )
                nc.sync.dma_start(out=logitsT_o[:, j0 : j0 + w], in_=lg[:])

    nc.compile()
    return nc


def kernel(**inputs):
    global LAST_EXEC_NS
    import ml_dtypes
    from concourse import bass_utils

    edge_index = np.asarray(inputs["edge_index"])
    edge_type = np.asarray(inputs["edge_type"])
    des = np.asarray(inputs["des"], dtype=np.float32)
    tweets = np.asarray(inputs["tweets"], dtype=np.float32)
    num = np.asarray(inputs["num"], dtype=np.float32)
    cat = np.asarray(inputs["cat"], dtype=np.float32)

    chunks, per_core = _preprocess(edge_index, edge_type)

    key = (chunks, "v3")
    if key not in _CACHE:
        _CACHE[key] = _build_program(chunks)
    nc = _CACHE[key]

    bf16 = ml_dtypes.bfloat16
    f32 = np.float32

    def padrows(a, rows):
        out = np.zeros((rows, a.shape[1]), dtype=a.dtype)
        out[: a.shape[0]] = a
        return out

    relw = np.asarray(inputs["rel_w"], dtype=f32)      # [2,2,D,D]
    rootw = np.asarray(inputs["root_w"], dtype=f32)    # [2,D,D]
    rgb = np.asarray(inputs["rgcn_b"], dtype=f32)      # [2,D]
    pbT = np.stack(
        [np.asarray(inputs[k], dtype=f32) for k in ("des_b", "tweet_b", "num_b", "cat_b")],
        axis=1,
    )  # [D, 4]
    iota = np.broadcast_to(np.arange(2 * W, dtype=f32), (P, 2 * W)).copy()
    ident = np.eye(P, dtype=f32)

    common = dict(
        desw_i=np.asarray(inputs["des_w"], dtype=f32).astype(bf16),
        tww_i=np.asarray(inputs["tweet_w"], dtype=f32).astype(bf16),
        numw_i=np.asarray(inputs["num_w"], dtype=f32),
        catw_i=np.asarray(inputs["cat_w"], dtype=f32),
        pbT_i=pbT,
        relw_i=relw.reshape(4 * D, D),
        rootw_i=rootw.reshape(2 * D, D),
        rgbT_i=rgb.T.copy(),
        m1w_i=np.asarray(inputs["mlp_w1"], dtype=f32),
        m1bT_i=np.asarray(inputs["mlp_b1"], dtype=f32).reshape(D, 1),
        m2w_i=np.asarray(inputs["mlp_w2"], dtype=f32),
        m2bT_i=np.asarray(inputs["mlp_b2"], dtype=f32).reshape(2, 1),
        iota_i=iota,
        ident_i=ident,
    )

    in_maps = []
    for c in range(NC):
        sl = slice(c * S, (c + 1) * S)
        pc = per_core[c]
        in_maps.append(
            dict(
                common,
                desT_i=np.ascontiguousarray(padrows(des[sl].astype(bf16), S_PAD).T),
                twT_i=np.ascontiguousarray(padrows(tweets[sl].astype(bf16), S_PAD).T),
                numT_i=np.ascontiguousarray(padrows(num[sl], S_PAD).T),
                catT_i=np.ascontiguousarray(padrows(cat[sl], S_PAD).T),
                offs_i=pc["offs"],
                cods_i=pc["cods"],
                invs_i=pc["invs"],
            )
        )

    trace = os.environ.get("KERNEL_TRACE", "0") == "1"
    res = bass_utils.run_bass_kernel_spmd(
        nc, in_maps, core_ids=list(range(NC)), trace=trace
    )
    LAST_EXEC_NS = res.exec_time_ns

    out = np.empty((N, 2), dtype=np.float32)
    for c in range(NC):
        lt = res.results[c]["logitsT_o"]  # [2, S_PAD]
        out[c * S : (c + 1) * S] = lt[:, :S].T
    return out


# revision 14
# speedup vs baseline: 1.0807x; 1.0567x over previous
"""BotRGCN Trainium2 kernel: 8-core SPMD bass kernel.

Strategy (graph/data parallel, per sharding hint):
- Nodes sharded: core c owns dst nodes [c*6250, (c+1)*6250).
- Multi-modal projection: each core computes x0 for its own nodes
  (des/tweets in bf16 via DMA-transpose + matmul, num/cat fp32).
- AllGather x across the 8 cores -> full node-feature table in DRAM.
- RGCN layer: edges (partitioned by dst) sorted by 32-dst windows;
  per 128-edge chunk: indirect-DMA gather of source rows, one-hot
  (iota==code) B matrix on DVE, TensorE matmul G^T.B accumulating
  aggT[fi, 64] per window (both relations side by side), scaled by
  host-precomputed 1/deg, then per-window epilogue matmuls
  (W_r^T @ aggT_r + root^T @ x_ownT) + leaky-relu -> x_{l+1}^T.
- MLP head on each core's own slice; host concatenates [N,2] output.
"""

import os
import numpy as np

NC = 8
N = 50000
S = 6250          # dst nodes per core
S_PAD = 6272      # = 49*128
NTBL = NC * S_PAD  # 50176
D = 128
W = 64            # dsts per aggregation window
NWIN = S_PAD // W  # 196
E = 1600000
DES = 768
TWE = 768
NUMF = 5
CATF = 6
P = 128
ZERO_ROW = S      # row 6250 of slice 0 is a zero pad row (global row index S)

LAST_EXEC_NS = None

_CACHE = {}


def _preprocess(edge_index, edge_type):
    """Sort/pad edges per core into fixed chunk structure shared by all cores.

    Returns (chunks, per_core) where chunks[w] = number of 128-edge chunks for
    window w (same for every core = max over cores), and per_core is a list of
    dicts with offs [128, ncalls] i32, codes [128, ncalls] f32,
    invc [1, NWIN*2*W] f32.
    """
    src = np.asarray(edge_index[0], dtype=np.int64)
    dst = np.asarray(edge_index[1], dtype=np.int64)
    et = np.asarray(edge_type, dtype=np.int64)

    core = dst // S
    dloc = dst - core * S
    win = dloc // W
    code = (dloc - win * W) + W * et          # 0..63
    gsrc = (src // S) * S_PAD + (src % S)     # global table row of src

    per_core_data = []
    cnts = np.zeros((NC, NWIN), dtype=np.int64)
    for c in range(NC):
        m = core == c
        w_c = win[m]
        g_c = gsrc[m]
        cd_c = code[m]
        d_c = dloc[m]
        r_c = et[m]
        order = np.lexsort((g_c, w_c))
        w_c, g_c, cd_c = w_c[order], g_c[order], cd_c[order]
        d_c, r_c = d_c[order], r_c[order]
        cnt = np.bincount(w_c, minlength=NWIN)
        cnts[c] = cnt
        # per (dloc, rel) in-degree for mean
        deg = np.bincount(d_c * 2 + r_c, minlength=S * 2).reshape(S, 2)
        per_core_data.append((g_c, cd_c, d_c, r_c, cnt, deg))

    chunks = ((cnts.max(axis=0) + 127) // 128).astype(np.int64)
    ncalls = int(chunks.sum())
    starts = np.concatenate([[0], np.cumsum(chunks)])

    per_core = []
    for c in range(NC):
        g_c, cd_c, d_c, r_c, cnt, deg = per_core_data[c]
        inv_e = (1.0 / np.maximum(deg[d_c, r_c], 1.0)).astype(np.float32)
        offs = np.full(ncalls * P, ZERO_ROW, dtype=np.int32)
        cods = np.zeros(ncalls * P, dtype=np.float32)
        invs = np.zeros(ncalls * P, dtype=np.float32)
        epos = np.concatenate([[0], np.cumsum(cnt)])
        for w in range(NWIN):
            n = int(cnt[w])
            if n == 0:
                continue
            base = int(starts[w]) * P
            offs[base : base + n] = g_c[epos[w] : epos[w + 1]]
            cods[base : base + n] = cd_c[epos[w] : epos[w + 1]]
            invs[base : base + n] = inv_e[epos[w] : epos[w + 1]]
        # [ncalls*P] -> [P, ncalls]: call k partition p = element k*P+p
        offs = offs.reshape(ncalls, P).T.copy()
        cods = cods.reshape(ncalls, P).T.copy()
        invs = invs.reshape(ncalls, P).T.copy()
        per_core.append(dict(offs=offs, cods=cods, invs=invs))
    return tuple(int(x) for x in chunks), per_core


def _build_program(chunks):
    import concourse.bass as bass
    import concourse.bacc as bacc
    import concourse.mybir as mybir
    import concourse.tile as tile

    f32 = mybir.dt.float32
    bf16 = mybir.dt.bfloat16
    i32 = mybir.dt.int32
    AF = mybir.ActivationFunctionType
    OP = mybir.AluOpType

    ncalls = int(sum(chunks))

    nc = bacc.Bacc("TRN2", target_bir_lowering=False, debug=False, num_devices=NC)

    def din(name, shape, dt):
        return nc.dram_tensor(name, shape, dt, kind="ExternalInput").ap()

    desT_i = din("desT_i", [DES, S_PAD], bf16)
    twT_i = din("twT_i", [TWE, S_PAD], bf16)
    numT_i = din("numT_i", [NUMF, S_PAD], f32)
    catT_i = din("catT_i", [CATF, S_PAD], f32)
    desw_i = din("desw_i", [DES, D], bf16)
    tww_i = din("tww_i", [TWE, D], bf16)
    numw_i = din("numw_i", [NUMF, D], f32)
    catw_i = din("catw_i", [CATF, D], f32)
    pbT_i = din("pbT_i", [D, 4], f32)          # proj biases per modality
    relw_i = din("relw_i", [2 * 2 * D, D], f32)  # (l,r) blocks of [fi,fo]
    rootw_i = din("rootw_i", [2 * D, D], f32)
    rgbT_i = din("rgbT_i", [D, 2], f32)
    m1w_i = din("m1w_i", [D, D], f32)
    m1bT_i = din("m1bT_i", [D, 1], f32)
    m2w_i = din("m2w_i", [D, 2], f32)
    m2bT_i = din("m2bT_i", [2, 1], f32)
    offs_i = din("offs_i", [P, ncalls], i32)
    cods_i = din("cods_i", [P, ncalls], f32)
    invs_i = din("invs_i", [P, ncalls], f32)
    iota_i = din("iota_i", [P, 2 * W], f32)
    ident_i = din("ident_i", [P, P], f32)

    logitsT_o = nc.dram_tensor("logitsT_o", [2, S_PAD], f32, kind="ExternalOutput").ap()

    # node tile widths for projection / MLP
    widths = [(i * 512, 512) for i in range(12)] + [(12 * 512, 128)]

    with tile.TileContext(nc) as tc:
        with (
            tc.tile_pool(name="const", bufs=1) as cp,
            tc.tile_pool(name="work", bufs=2) as wp,
            tc.tile_pool(name="psum", bufs=1, space="PSUM") as pp,
            tc.tile_pool(name="dram", bufs=1, space="DRAM") as dp,
        ):
            # ---- constants to SBUF ----
            def cload(ap_in, shape, dt, name):
                t = cp.tile(shape, dt, name=name)
                nc.sync.dma_start(out=t[:], in_=ap_in)
                return t

            desw_c = [cload(desw_i[k * P : (k + 1) * P, :], [P, D], bf16, f"desw{k}") for k in range(6)]
            tww_c = [cload(tww_i[k * P : (k + 1) * P, :], [P, D], bf16, f"tww{k}") for k in range(6)]
            numw_c = cload(numw_i[:], [NUMF, D], f32, "numw")
            catw_c = cload(catw_i[:], [CATF, D], f32, "catw")
            pbT_c = cload(pbT_i[:], [D, 4], f32, "pbT")
            relw_c = [[cload(relw_i[(l * 2 + r) * D : (l * 2 + r + 1) * D, :], [P, D], f32, f"relw{l}{r}") for r in range(2)] for l in range(2)]
            rootw_c = [cload(rootw_i[l * D : (l + 1) * D, :], [P, D], f32, f"rootw{l}") for l in range(2)]
            rgbT_c = cload(rgbT_i[:], [D, 2], f32, "rgbT")
            m1w_c = cload(m1w_i[:], [D, D], f32, "m1w")
            m1bT_c = cload(m1bT_i[:], [D, 1], f32, "m1bT")
            m2w_c = cload(m2w_i[:], [D, 2], f32, "m2w")
            m2bT_c = cload(m2bT_i[:], [2, 1], f32, "m2bT")
            offs_c = cp.tile([P, ncalls], i32, name="offs")
            nc.sync.dma_start(out=offs_c[:], in_=offs_i[:])
            cods_c = cp.tile([P, ncalls], f32, name="cods")
            nc.sync.dma_start(out=cods_c[:], in_=cods_i[:])
            invs_c = cp.tile([P, ncalls], f32, name="invs")
            nc.sync.dma_start(out=invs_c[:], in_=invs_i[:])
            iota_c = cload(iota_i[:], [P, 2 * W], f32, "iota")
            ident_c = cload(ident_i[:], [P, P], f32, "ident")
            zeros_c = cp.tile([S_PAD - S, D], f32, name="zeros")
            nc.vector.memset(zeros_c[:], 0.0)

            # resident transposed activations
            x0T = cp.tile([P, S_PAD], f32, name="x0T")
            x1T = cp.tile([P, S_PAD], f32, name="x1T")
            x2T = cp.tile([P, S_PAD], f32, name="x2T")

            # internal DRAM
            ag_in = [dp.tile([S_PAD, D], f32, name=f"ag_in{l}") for l in range(2)]
            tbl = [dp.tile([NTBL, D], f32, name=f"tbl{l}", addr_space="Shared") for l in range(2)]

            # ---- Stage 1: projection -> x0T ----
            for (j0, w) in widths:
                acc = wp.tile([P, w], f32, tag="projacc", bufs=2)
                for mi, (dataT, wgts) in enumerate([(desT_i, desw_c), (twT_i, tww_c)]):
                    ps = pp.tile([P, w], f32, tag="proj", bufs=2)
                    for k in range(6):
                        dT = wp.tile([P, w], bf16, tag="dT", bufs=3)
                        nc.sync.dma_start(
                            out=dT[:],
                            in_=dataT[k * P : (k + 1) * P, j0 : j0 + w],
                        )
                        nc.tensor.matmul(
                            out=ps[:], lhsT=wgts[k][:], rhs=dT[:],
                            start=(k == 0), stop=(k == 5),
                        )
                    if mi == 0:
                        nc.scalar.activation(
                            out=acc[:], in_=ps[:], func=AF.Lrelu,
                            bias=pbT_c[:, mi : mi + 1], scale=1.0, alpha=0.01,
                        )
                    else:
                        t = wp.tile([P, w], f32, tag="projt", bufs=2)
                        nc.scalar.activation(
                            out=t[:], in_=ps[:], func=AF.Lrelu,
                            bias=pbT_c[:, mi : mi + 1], scale=1.0, alpha=0.01,
                        )
                        nc.vector.tensor_tensor(out=acc[:], in0=acc[:], in1=t[:], op=OP.add)
                for mi, (dataT, wgt, nk) in enumerate(
                    [(numT_i, numw_c, NUMF), (catT_i, catw_c, CATF)]
                ):
                    ps = pp.tile([P, w], f32, tag="proj", bufs=2)
                    rT = wp.tile([nk, w], f32, tag=f"rT{mi}", bufs=2)
                    nc.sync.dma_start(out=rT[:], in_=dataT[:, j0 : j0 + w])
                    nc.tensor.matmul(out=ps[:], lhsT=wgt[:nk, :], rhs=rT[:], start=True, stop=True)
                    t = wp.tile([P, w], f32, tag="projt", bufs=2)
                    nc.scalar.activation(
                        out=t[:], in_=ps[:], func=AF.Lrelu,
                        bias=pbT_c[:, 2 + mi : 3 + mi], scale=1.0, alpha=0.01,
                    )
                    nc.vector.tensor_tensor(out=acc[:], in0=acc[:], in1=t[:], op=OP.add)
                nc.vector.tensor_copy(out=x0T[:, j0 : j0 + w], in_=acc[:])

            # ---- helper: write xT -> node-major table slice + AllGather ----
            def export_and_allgather(xT, l):
                for t in range(49):
                    ps = pp.tile([P, P], f32, tag="tr", bufs=1)
                    nc.tensor.transpose(
                        out=ps[:], in_=xT[:, t * P : (t + 1) * P], identity=ident_c[:]
                    )
                    xx = wp.tile([P, P], f32, tag="xrow", bufs=2)
                    nc.scalar.activation(out=xx[:], in_=ps[:], func=AF.Copy)
                    if t < 48:
                        nc.sync.dma_start(out=ag_in[l][t * P : (t + 1) * P, :], in_=xx[:])
                    else:
                        nc.sync.dma_start(out=ag_in[l][48 * P : S, :], in_=xx[: S - 48 * P, :])
                nc.sync.dma_start(out=ag_in[l][S:S_PAD, :], in_=zeros_c[:])
                nc.gpsimd.collective_compute(
                    "AllGather",
                    mybir.AluOpType.bypass,
                    replica_groups=[list(range(NC))],
                    ins=[ag_in[l].opt()],
                    outs=[tbl[l].opt()],
                )

            # ---- RGCN layer ----
            def layer(l, xT_in, xT_out):
                gk = 0
                for w in range(NWIN):
                    nch = chunks[w]
                    if nch:
                        agg_ps = pp.tile([P, 2 * W], f32, tag="agg", bufs=2)
                        for k in range(nch):
                            g = wp.tile([P, D], f32, tag="g", bufs=24)
                            nc.gpsimd.indirect_dma_start(
                                out=g[:],
                                out_offset=None,
                                in_=tbl[l][:],
                                in_offset=bass.IndirectOffsetOnAxis(
                                    ap=offs_c[:, gk : gk + 1], axis=0
                                ),
                            )
                            B = wp.tile([P, 2 * W], f32, tag="B", bufs=24)
                            nc.vector.tensor_scalar(
                                out=B[:], in0=iota_c[:],
                                scalar1=cods_c[:, gk : gk + 1],
                                scalar2=invs_c[:, gk : gk + 1],
                                op0=OP.is_equal, op1=OP.mult,
                            )
                            nc.tensor.matmul(
                                out=agg_ps[:], lhsT=g[:], rhs=B[:],
                                start=(k == 0), stop=(k == nch - 1),
                            )
                            gk += 1
                        aggT = wp.tile([P, 2 * W], f32, tag="aggs", bufs=2)
                        nc.vector.tensor_copy(out=aggT[:], in_=agg_ps[:])
                    out_ps = pp.tile([P, W], f32, tag="outw", bufs=2)
                    nc.tensor.matmul(
                        out=out_ps[:], lhsT=rootw_c[l][:],
                        rhs=xT_in[:, w * W : (w + 1) * W],
                        start=True, stop=(nch == 0),
                    )
                    if nch:
                        nc.tensor.matmul(
                            out=out_ps[:], lhsT=relw_c[l][0][:], rhs=aggT[:, 0:W],
                            start=False, stop=False,
                        )
                        nc.tensor.matmul(
                            out=out_ps[:], lhsT=relw_c[l][1][:], rhs=aggT[:, W : 2 * W],
                            start=False, stop=True,
                        )
                    nc.scalar.activation(
                        out=xT_out[:, w * W : (w + 1) * W], in_=out_ps[:],
                        func=AF.Lrelu, bias=rgbT_c[:, l : l + 1], scale=1.0, alpha=0.01,
                    )

            export_and_allgather(x0T, 0)
            layer(0, x0T, x1T)
            export_and_allgather(x1T, 1)
            layer(1, x1T, x2T)

            # ---- MLP head ----
            for (j0, w) in widths:
                ps = pp.tile([P, w], f32, tag="proj", bufs=2)
                nc.tensor.matmul(out=ps[:], lhsT=m1w_c[:], rhs=x2T[:, j0 : j0 + w], start=True, stop=True)
                hT = wp.tile([P, w], f32, tag="hT", bufs=2)
                nc.scalar.activation(out=hT[:], in_=ps[:], func=AF.Relu, bias=m1bT_c[:, 0:1])
                ps2 = pp.tile([2, w], f32, tag="mlp2", bufs=1)
                nc.tensor.matmul(out=ps2[:], lhsT=m2w_c[:], rhs=hT[:], start=True, stop=True)
                lg = wp.tile([2, w], f32, tag="lg", bufs=2)
                nc.vector.tensor_scalar(
                    out=lg[:], in0=ps2[:], scalar1=m2bT_c[:, 0:1], scalar2=None, op0=OP.add
                )
                nc.sync.dma_start(out=logitsT_o[:, j0 : j0 + w], in_=lg[:])

    nc.compile()
    return nc


def kernel(**inputs):
    global LAST_EXEC_NS
    import ml_dtypes
    from concourse import bass_utils

    edge_index = np.asarray(inputs["edge_index"])
    edge_type = np.asarray(inputs["edge_type"])
    des = np.asarray(inputs["des"], dtype=np.float32)
    tweets = np.asarray(inputs["tweets"], dtype=np.float32)
    num = np.asarray(inputs["num"], dtype=np.float32)
    cat = np.asarray(inputs["cat"], dtype=np.float32)

    chunks, per_core = _preprocess(edge_index, edge_type)

    key = (chunks, "v4")
    if key not in _CACHE:
        _CACHE[key] = _build_program(chunks)
    nc = _CACHE[key]

    bf16 = ml_dtypes.bfloat16
    f32 = np.float32

    def padrows(a, rows):
        out = np.zeros((rows, a.shape[1]), dtype=a.dtype)
        out[: a.shape[0]] = a
        return out

    relw = np.asarray(inputs["rel_w"], dtype=f32)      # [2,2,D,D]
    rootw = np.asarray(inputs["root_w"], dtype=f32)    # [2,D,D]
    rgb = np.asarray(inputs["rgcn_b"], dtype=f32)      # [2,D]
    pbT = np.stack(
        [np.asarray(inputs[k], dtype=f32) for k in ("des_b", "tweet_b", "num_b", "cat_b")],
        axis=1,
    )  # [D, 4]
    iota = np.broadcast_to(np.arange(2 * W, dtype=f32), (P, 2 * W)).copy()
    ident = np.eye(P, dtype=f32)

    common = dict(
        desw_i=np.asarray(inputs["des_w"], dtype=f32).astype(bf16),
        tww_i=np.asarray(inputs["tweet_w"], dtype=f32).astype(bf16),
        numw_i=np.asarray(inputs["num_w"], dtype=f32),
        catw_i=np.asarray(inputs["cat_w"], dtype=f32),
        pbT_i=pbT,
        relw_i=relw.reshape(4 * D, D),
        rootw_i=rootw.reshape(2 * D, D),
        rgbT_i=rgb.T.copy(),
        m1w_i=np.asarray(inputs["mlp_w1"], dtype=f32),
        m1bT_i=np.asarray(inputs["mlp_b1"], dtype=f32).reshape(D, 1),
        m2w_i=np.asarray(inputs["mlp_w2"], dtype=f32),
        m2bT_i=np.asarray(inputs["mlp_b2"], dtype=f32).reshape(2, 1),
        iota_i=iota,
        ident_i=ident,
    )

    in_maps = []
    for c in range(NC):
        sl = slice(c * S, (c + 1) * S)
        pc = per_core[c]
        in_maps.append(
            dict(
                common,
                desT_i=np.ascontiguousarray(padrows(des[sl].astype(bf16), S_PAD).T),
                twT_i=np.ascontiguousarray(padrows(tweets[sl].astype(bf16), S_PAD).T),
                numT_i=np.ascontiguousarray(padrows(num[sl], S_PAD).T),
                catT_i=np.ascontiguousarray(padrows(cat[sl], S_PAD).T),
                offs_i=pc["offs"],
                cods_i=pc["cods"],
                invs_i=pc["invs"],
            )
        )

    trace = os.environ.get("KERNEL_TRACE", "0") == "1"
    res = bass_utils.run_bass_kernel_spmd(
        nc, in_maps, core_ids=list(range(NC)), trace=trace
    )
    LAST_EXEC_NS = res.exec_time_ns

    out = np.empty((N, 2), dtype=np.float32)
    for c in range(NC):
        lt = res.results[c]["logitsT_o"]  # [2, S_PAD]
        out[c * S : (c + 1) * S] = lt[:, :S].T
    return out


# revision 15
# speedup vs baseline: 1.0934x; 1.0117x over previous
"""BotRGCN Trainium2 kernel: 8-core SPMD bass kernel.

Strategy (graph/data parallel, per sharding hint):
- Nodes sharded: core c owns dst nodes [c*6250, (c+1)*6250).
- Multi-modal projection: each core computes x0 for its own nodes
  (des/tweets in bf16 via DMA-transpose + matmul, num/cat fp32).
- AllGather x across the 8 cores -> full node-feature table in DRAM.
- RGCN layer: edges (partitioned by dst) sorted by 32-dst windows;
  per 128-edge chunk: indirect-DMA gather of source rows, one-hot
  (iota==code) B matrix on DVE, TensorE matmul G^T.B accumulating
  aggT[fi, 64] per window (both relations side by side), scaled by
  host-precomputed 1/deg, then per-window epilogue matmuls
  (W_r^T @ aggT_r + root^T @ x_ownT) + leaky-relu -> x_{l+1}^T.
- MLP head on each core's own slice; host concatenates [N,2] output.
"""

import os
import numpy as np

NC = 8
N = 50000
S = 6250          # dst nodes per core
S_PAD = 6272      # = 49*128
NTBL = NC * S_PAD  # 50176
D = 128
W = 128           # dsts per aggregation window
NWIN = S_PAD // W  # 196
E = 1600000
DES = 768
TWE = 768
NUMF = 5
CATF = 6
P = 128
ZERO_ROW = S      # row 6250 of slice 0 is a zero pad row (global row index S)

LAST_EXEC_NS = None

_CACHE = {}


def _preprocess(edge_index, edge_type):
    """Sort/pad edges per core into fixed chunk structure shared by all cores.

    Returns (chunks, per_core) where chunks[w] = number of 128-edge chunks for
    window w (same for every core = max over cores), and per_core is a list of
    dicts with offs [128, ncalls] i32, codes [128, ncalls] f32,
    invc [1, NWIN*2*W] f32.
    """
    src = np.asarray(edge_index[0], dtype=np.int64)
    dst = np.asarray(edge_index[1], dtype=np.int64)
    et = np.asarray(edge_type, dtype=np.int64)

    core = dst // S
    dloc = dst - core * S
    win = dloc // W
    code = (dloc - win * W) + W * et          # 0..63
    gsrc = (src // S) * S_PAD + (src % S)     # global table row of src

    per_core_data = []
    cnts = np.zeros((NC, NWIN), dtype=np.int64)
    for c in range(NC):
        m = core == c
        w_c = win[m]
        g_c = gsrc[m]
        cd_c = code[m]
        d_c = dloc[m]
        r_c = et[m]
        order = np.lexsort((g_c, w_c))
        w_c, g_c, cd_c = w_c[order], g_c[order], cd_c[order]
        d_c, r_c = d_c[order], r_c[order]
        cnt = np.bincount(w_c, minlength=NWIN)
        cnts[c] = cnt
        # per (dloc, rel) in-degree for mean
        deg = np.bincount(d_c * 2 + r_c, minlength=S * 2).reshape(S, 2)
        per_core_data.append((g_c, cd_c, d_c, r_c, cnt, deg))

    chunks = ((cnts.max(axis=0) + 127) // 128).astype(np.int64)
    ncalls = int(chunks.sum())
    starts = np.concatenate([[0], np.cumsum(chunks)])

    per_core = []
    for c in range(NC):
        g_c, cd_c, d_c, r_c, cnt, deg = per_core_data[c]
        inv_e = (1.0 / np.maximum(deg[d_c, r_c], 1.0)).astype(np.float32)
        offs = np.full(ncalls * P, ZERO_ROW, dtype=np.int32)
        cods = np.zeros(ncalls * P, dtype=np.float32)
        invs = np.zeros(ncalls * P, dtype=np.float32)
        epos = np.concatenate([[0], np.cumsum(cnt)])
        for w in range(NWIN):
            n = int(cnt[w])
            if n == 0:
                continue
            base = int(starts[w]) * P
            offs[base : base + n] = g_c[epos[w] : epos[w + 1]]
            cods[base : base + n] = cd_c[epos[w] : epos[w + 1]]
            invs[base : base + n] = inv_e[epos[w] : epos[w + 1]]
        # [ncalls*P] -> [P, ncalls]: call k partition p = element k*P+p
        offs = offs.reshape(ncalls, P).T.copy()
        cods = cods.reshape(ncalls, P).T.copy()
        invs = invs.reshape(ncalls, P).T.copy()
        per_core.append(dict(offs=offs, cods=cods, invs=invs))
    return tuple(int(x) for x in chunks), per_core


def _build_program(chunks):
    import concourse.bass as bass
    import concourse.bacc as bacc
    import concourse.mybir as mybir
    import concourse.tile as tile

    f32 = mybir.dt.float32
    bf16 = mybir.dt.bfloat16
    i32 = mybir.dt.int32
    AF = mybir.ActivationFunctionType
    OP = mybir.AluOpType

    ncalls = int(sum(chunks))

    nc = bacc.Bacc("TRN2", target_bir_lowering=False, debug=False, num_devices=NC)

    def din(name, shape, dt):
        return nc.dram_tensor(name, shape, dt, kind="ExternalInput").ap()

    desT_i = din("desT_i", [DES, S_PAD], bf16)
    twT_i = din("twT_i", [TWE, S_PAD], bf16)
    numT_i = din("numT_i", [NUMF, S_PAD], f32)
    catT_i = din("catT_i", [CATF, S_PAD], f32)
    desw_i = din("desw_i", [DES, D], bf16)
    tww_i = din("tww_i", [TWE, D], bf16)
    numw_i = din("numw_i", [NUMF, D], f32)
    catw_i = din("catw_i", [CATF, D], f32)
    pbT_i = din("pbT_i", [D, 4], f32)          # proj biases per modality
    relw_i = din("relw_i", [2 * 2 * D, D], f32)  # (l,r) blocks of [fi,fo]
    rootw_i = din("rootw_i", [2 * D, D], f32)
    rgbT_i = din("rgbT_i", [D, 2], f32)
    m1w_i = din("m1w_i", [D, D], f32)
    m1bT_i = din("m1bT_i", [D, 1], f32)
    m2w_i = din("m2w_i", [D, 2], f32)
    m2bT_i = din("m2bT_i", [2, 1], f32)
    offs_i = din("offs_i", [P, ncalls], i32)
    cods_i = din("cods_i", [P, ncalls], f32)
    invs_i = din("invs_i", [P, ncalls], f32)
    iota_i = din("iota_i", [P, 2 * W], f32)
    ident_i = din("ident_i", [P, P], f32)

    logitsT_o = nc.dram_tensor("logitsT_o", [2, S_PAD], f32, kind="ExternalOutput").ap()

    # node tile widths for projection / MLP
    widths = [(i * 512, 512) for i in range(12)] + [(12 * 512, 128)]

    with tile.TileContext(nc) as tc:
        with (
            tc.tile_pool(name="const", bufs=1) as cp,
            tc.tile_pool(name="work", bufs=2) as wp,
            tc.tile_pool(name="psum", bufs=1, space="PSUM") as pp,
            tc.tile_pool(name="dram", bufs=1, space="DRAM") as dp,
        ):
            # ---- constants to SBUF ----
            def cload(ap_in, shape, dt, name):
                t = cp.tile(shape, dt, name=name)
                nc.sync.dma_start(out=t[:], in_=ap_in)
                return t

            desw_c = [cload(desw_i[k * P : (k + 1) * P, :], [P, D], bf16, f"desw{k}") for k in range(6)]
            tww_c = [cload(tww_i[k * P : (k + 1) * P, :], [P, D], bf16, f"tww{k}") for k in range(6)]
            numw_c = cload(numw_i[:], [NUMF, D], f32, "numw")
            catw_c = cload(catw_i[:], [CATF, D], f32, "catw")
            pbT_c = cload(pbT_i[:], [D, 4], f32, "pbT")
            relw_c = [[cload(relw_i[(l * 2 + r) * D : (l * 2 + r + 1) * D, :], [P, D], f32, f"relw{l}{r}") for r in range(2)] for l in range(2)]
            rootw_c = [cload(rootw_i[l * D : (l + 1) * D, :], [P, D], f32, f"rootw{l}") for l in range(2)]
            rgbT_c = cload(rgbT_i[:], [D, 2], f32, "rgbT")
            m1w_c = cload(m1w_i[:], [D, D], f32, "m1w")
            m1bT_c = cload(m1bT_i[:], [D, 1], f32, "m1bT")
            m2w_c = cload(m2w_i[:], [D, 2], f32, "m2w")
            m2bT_c = cload(m2bT_i[:], [2, 1], f32, "m2bT")
            offs_c = cp.tile([P, ncalls], i32, name="offs")
            nc.sync.dma_start(out=offs_c[:], in_=offs_i[:])
            cods_c = cp.tile([P, ncalls], f32, name="cods")
            nc.sync.dma_start(out=cods_c[:], in_=cods_i[:])
            invs_c = cp.tile([P, ncalls], f32, name="invs")
            nc.sync.dma_start(out=invs_c[:], in_=invs_i[:])
            iota_c = cload(iota_i[:], [P, 2 * W], f32, "iota")
            ident_c = cload(ident_i[:], [P, P], f32, "ident")
            zeros_c = cp.tile([S_PAD - S, D], f32, name="zeros")
            nc.vector.memset(zeros_c[:], 0.0)

            # resident transposed activations
            x0T = cp.tile([P, S_PAD], f32, name="x0T")
            x1T = cp.tile([P, S_PAD], f32, name="x1T")
            x2T = cp.tile([P, S_PAD], f32, name="x2T")

            # internal DRAM
            ag_in = [dp.tile([S_PAD, D], f32, name=f"ag_in{l}") for l in range(2)]
            tbl = [dp.tile([NTBL, D], f32, name=f"tbl{l}", addr_space="Shared") for l in range(2)]

            # ---- Stage 1: projection -> x0T ----
            for (j0, w) in widths:
                acc = wp.tile([P, w], f32, tag="projacc", bufs=2)
                for mi, (dataT, wgts) in enumerate([(desT_i, desw_c), (twT_i, tww_c)]):
                    ps = pp.tile([P, w], f32, tag="proj", bufs=2)
                    for k in range(6):
                        dT = wp.tile([P, w], bf16, tag="dT", bufs=3)
                        nc.sync.dma_start(
                            out=dT[:],
                            in_=dataT[k * P : (k + 1) * P, j0 : j0 + w],
                        )
                        nc.tensor.matmul(
                            out=ps[:], lhsT=wgts[k][:], rhs=dT[:],
                            start=(k == 0), stop=(k == 5),
                        )
                    if mi == 0:
                        nc.scalar.activation(
                            out=acc[:], in_=ps[:], func=AF.Lrelu,
                            bias=pbT_c[:, mi : mi + 1], scale=1.0, alpha=0.01,
                        )
                    else:
                        t = wp.tile([P, w], f32, tag="projt", bufs=2)
                        nc.scalar.activation(
                            out=t[:], in_=ps[:], func=AF.Lrelu,
                            bias=pbT_c[:, mi : mi + 1], scale=1.0, alpha=0.01,
                        )
                        nc.vector.tensor_tensor(out=acc[:], in0=acc[:], in1=t[:], op=OP.add)
                for mi, (dataT, wgt, nk) in enumerate(
                    [(numT_i, numw_c, NUMF), (catT_i, catw_c, CATF)]
                ):
                    ps = pp.tile([P, w], f32, tag="proj", bufs=2)
                    rT = wp.tile([nk, w], f32, tag=f"rT{mi}", bufs=2)
                    nc.sync.dma_start(out=rT[:], in_=dataT[:, j0 : j0 + w])
                    nc.tensor.matmul(out=ps[:], lhsT=wgt[:nk, :], rhs=rT[:], start=True, stop=True)
                    t = wp.tile([P, w], f32, tag="projt", bufs=2)
                    nc.scalar.activation(
                        out=t[:], in_=ps[:], func=AF.Lrelu,
                        bias=pbT_c[:, 2 + mi : 3 + mi], scale=1.0, alpha=0.01,
                    )
                    nc.vector.tensor_tensor(out=acc[:], in0=acc[:], in1=t[:], op=OP.add)
                nc.vector.tensor_copy(out=x0T[:, j0 : j0 + w], in_=acc[:])

            # ---- helper: write xT -> node-major table slice + AllGather ----
            def export_and_allgather(xT, l):
                for t in range(49):
                    ps = pp.tile([P, P], f32, tag="tr", bufs=1)
                    nc.tensor.transpose(
                        out=ps[:], in_=xT[:, t * P : (t + 1) * P], identity=ident_c[:]
                    )
                    xx = wp.tile([P, P], f32, tag="xrow", bufs=2)
                    nc.scalar.activation(out=xx[:], in_=ps[:], func=AF.Copy)
                    if t < 48:
                        nc.sync.dma_start(out=ag_in[l][t * P : (t + 1) * P, :], in_=xx[:])
                    else:
                        nc.sync.dma_start(out=ag_in[l][48 * P : S, :], in_=xx[: S - 48 * P, :])
                nc.sync.dma_start(out=ag_in[l][S:S_PAD, :], in_=zeros_c[:])
                nc.gpsimd.collective_compute(
                    "AllGather",
                    mybir.AluOpType.bypass,
                    replica_groups=[list(range(NC))],
                    ins=[ag_in[l].opt()],
                    outs=[tbl[l].opt()],
                )

            # ---- RGCN layer ----
            def layer(l, xT_in, xT_out):
                gk = 0
                for w in range(NWIN):
                    nch = chunks[w]
                    if nch:
                        agg_ps = pp.tile([P, 2 * W], f32, tag="agg", bufs=2)
                        for k in range(nch):
                            g = wp.tile([P, D], f32, tag="g", bufs=24)
                            nc.gpsimd.indirect_dma_start(
                                out=g[:],
                                out_offset=None,
                                in_=tbl[l][:],
                                in_offset=bass.IndirectOffsetOnAxis(
                                    ap=offs_c[:, gk : gk + 1], axis=0
                                ),
                            )
                            B = wp.tile([P, 2 * W], f32, tag="B", bufs=24)
                            nc.vector.tensor_scalar(
                                out=B[:], in0=iota_c[:],
                                scalar1=cods_c[:, gk : gk + 1],
                                scalar2=invs_c[:, gk : gk + 1],
                                op0=OP.is_equal, op1=OP.mult,
                            )
                            nc.tensor.matmul(
                                out=agg_ps[:], lhsT=g[:], rhs=B[:],
                                start=(k == 0), stop=(k == nch - 1),
                            )
                            gk += 1
                        aggT = wp.tile([P, 2 * W], f32, tag="aggs", bufs=2)
                        nc.vector.tensor_copy(out=aggT[:], in_=agg_ps[:])
                    out_ps = pp.tile([P, W], f32, tag="outw", bufs=2)
                    nc.tensor.matmul(
                        out=out_ps[:], lhsT=rootw_c[l][:],
                        rhs=xT_in[:, w * W : (w + 1) * W],
                        start=True, stop=(nch == 0),
                    )
                    if nch:
                        nc.tensor.matmul(
                            out=out_ps[:], lhsT=relw_c[l][0][:], rhs=aggT[:, 0:W],
                            start=False, stop=False,
                        )
                        nc.tensor.matmul(
                            out=out_ps[:], lhsT=relw_c[l][1][:], rhs=aggT[:, W : 2 * W],
                            start=False, stop=True,
                        )
                    nc.scalar.activation(
                        out=xT_out[:, w * W : (w + 1) * W], in_=out_ps[:],
                        func=AF.Lrelu, bias=rgbT_c[:, l : l + 1], scale=1.0, alpha=0.01,
                    )

            export_and_allgather(x0T, 0)
            layer(0, x0T, x1T)
            export_and_allgather(x1T, 1)
            layer(1, x1T, x2T)

            # ---- MLP head ----
            for (j0, w) in widths:
                ps = pp.tile([P, w], f32, tag="proj", bufs=2)
                nc.tensor.matmul(out=ps[:], lhsT=m1w_c[:], rhs=x2T[:, j0 : j0 + w], start=True, stop=True)
                hT = wp.tile([P, w], f32, tag="hT", bufs=2)
                nc.scalar.activation(out=hT[:], in_=ps[:], func=AF.Relu, bias=m1bT_c[:, 0:1])
                ps2 = pp.tile([2, w], f32, tag="mlp2", bufs=1)
                nc.tensor.matmul(out=ps2[:], lhsT=m2w_c[:], rhs=hT[:], start=True, stop=True)
                lg = wp.tile([2, w], f32, tag="lg", bufs=2)
                nc.vector.tensor_scalar(
                    out=lg[:], in0=ps2[:], scalar1=m2bT_c[:, 0:1], scalar2=None, op0=OP.add
                )
                nc.sync.dma_start(out=logitsT_o[:, j0 : j0 + w], in_=lg[:])

    nc.compile()
    return nc


def kernel(**inputs):
    global LAST_EXEC_NS
    import ml_dtypes
    from concourse import bass_utils

    edge_index = np.asarray(inputs["edge_index"])
    edge_type = np.asarray(inputs["edge_type"])
    des = np.asarray(inputs["des"], dtype=np.float32)
    tweets = np.asarray(inputs["tweets"], dtype=np.float32)
    num = np.asarray(inputs["num"], dtype=np.float32)
    cat = np.asarray(inputs["cat"], dtype=np.float32)

    chunks, per_core = _preprocess(edge_index, edge_type)

    key = (chunks, "v5")
    if key not in _CACHE:
        _CACHE[key] = _build_program(chunks)
    nc = _CACHE[key]

    bf16 = ml_dtypes.bfloat16
    f32 = np.float32

    def padrows(a, rows):
        out = np.zeros((rows, a.shape[1]), dtype=a.dtype)
        out[: a.shape[0]] = a
        return out

    relw = np.asarray(inputs["rel_w"], dtype=f32)      # [2,2,D,D]
    rootw = np.asarray(inputs["root_w"], dtype=f32)    # [2,D,D]
    rgb = np.asarray(inputs["rgcn_b"], dtype=f32)      # [2,D]
    pbT = np.stack(
        [np.asarray(inputs[k], dtype=f32) for k in ("des_b", "tweet_b", "num_b", "cat_b")],
        axis=1,
    )  # [D, 4]
    iota = np.broadcast_to(np.arange(2 * W, dtype=f32), (P, 2 * W)).copy()
    ident = np.eye(P, dtype=f32)

    common = dict(
        desw_i=np.asarray(inputs["des_w"], dtype=f32).astype(bf16),
        tww_i=np.asarray(inputs["tweet_w"], dtype=f32).astype(bf16),
        numw_i=np.asarray(inputs["num_w"], dtype=f32),
        catw_i=np.asarray(inputs["cat_w"], dtype=f32),
        pbT_i=pbT,
        relw_i=relw.reshape(4 * D, D),
        rootw_i=rootw.reshape(2 * D, D),
        rgbT_i=rgb.T.copy(),
        m1w_i=np.asarray(inputs["mlp_w1"], dtype=f32),
        m1bT_i=np.asarray(inputs["mlp_b1"], dtype=f32).reshape(D, 1),
        m2w_i=np.asarray(inputs["mlp_w2"], dtype=f32),
        m2bT_i=np.asarray(inputs["mlp_b2"], dtype=f32).reshape(2, 1),
        iota_i=iota,
        ident_i=ident,
    )

    in_maps = []
    for c in range(NC):
        sl = slice(c * S, (c + 1) * S)
        pc = per_core[c]
        in_maps.append(
            dict(
                common,
                desT_i=np.ascontiguousarray(padrows(des[sl].astype(bf16), S_PAD).T),
                twT_i=np.ascontiguousarray(padrows(tweets[sl].astype(bf16), S_PAD).T),
                numT_i=np.ascontiguousarray(padrows(num[sl], S_PAD).T),
                catT_i=np.ascontiguousarray(padrows(cat[sl], S_PAD).T),
                offs_i=pc["offs"],
                cods_i=pc["cods"],
                invs_i=pc["invs"],
            )
        )

    trace = os.environ.get("KERNEL_TRACE", "0") == "1"
    res = bass_utils.run_bass_kernel_spmd(
        nc, in_maps, core_ids=list(range(NC)), trace=trace
    )
    LAST_EXEC_NS = res.exec_time_ns

    out = np.empty((N, 2), dtype=np.float32)
    for c in range(NC):
        lt = res.results[c]["logitsT_o"]  # [2, S_PAD]
        out[c * S : (c + 1) * S] = lt[:, :S].T
    return out
